# revision 10
# baseline (speedup 1.0000x reference)
"""CQAttention Trainium2 kernel: out = concat([C, A, C*A, C*Bv], -1).

Math notes (exact, not approximations):
  - similarity = sub0 + sub1 + sub2 + bias is consumed only through
    softmax over the last axis (m). sub0 (per-row) and bias (scalar) are
    constant along m, so they cancel in both softmaxes -> dropped.
  - With all-ones masks S1 == S2, so Bt = S1 @ S1^T.
  - Softmax normalization is folded into per-partition scales:
      E^T[m,n] = exp(sub1[m] + sub2[n,m])   (no max-subtract; |scores| ~ 12)
      r[n] = 1 / sum_m E[n,m]
      A  = diag(r) (E Q)
      F  = E E^T (symmetric);  F'' = diag(r) F
      Bv = diag(r) (F''^T contracted with C) = diag(r) (sum_k F''[k,n] C[k,:])
  - All matmuls run as float32r (full-rate fp32 on trn2 PE).

Sharding: data-parallel over batch; core i handles batches [2i, 2i+1].
"""

import sys

if "/opt/trn_rl_repo" not in sys.path:
    sys.path.insert(0, "/opt/trn_rl_repo")

import numpy as np

B, N, M, D = 16, 1024, 512, 512
NCORES = 8
BPC = B // NCORES  # batches per core
P = 128
NC = N // P  # 8 n-chunks
MC = M // P  # 4 m-chunks
DC = D // P  # 4 d-chunks

_cache = {}


def _split_drain_waits(nc, max_waits=1):
    """This container's walrus rejects Drain instructions carrying more than
    ~1 sem wait. Hoist extra waits onto preceding single-wait Drains."""
    from concourse import mybir

    for f in nc.m.functions:
        for blk in f.blocks:
            out = []
            changed = False
            for inst in blk.instructions:
                si = inst.sync_info
                if (
                    getattr(inst, "opcode", None) == "Drain"
                    and si is not None
                    and si.on_wait is not None
                    and len(si.on_wait) > max_waits
                ):
                    waits = list(si.on_wait)
                    head, tail = waits[:-max_waits], waits[-max_waits:]
                    for j, w in enumerate(head):
                        out.append(
                            mybir.InstDrain(
                                name=f"{inst.name}-sw{j}",
                                engine=inst.engine,
                                ins=[],
                                outs=[],
                                sync_info=mybir.SyncInfo(on_wait=[w], on_update=[]),
                            )
                        )
                    inst.sync_info = mybir.SyncInfo(
                        on_wait=tail, on_update=list(si.on_update or [])
                    )
                    changed = True
                out.append(inst)
            if changed:
                blk.instructions = out


def _build(phase=70):
    import concourse.bass as bass
    import concourse.tile as tile
    from concourse import bacc, mybir
    from concourse.masks import make_identity

    f32 = mybir.dt.float32
    f32r = mybir.dt.float32r
    ACT = mybir.ActivationFunctionType

    nc = bacc.Bacc("TRN2")
    Cd = nc.dram_tensor("C", [BPC, N, D], f32, kind="ExternalInput")
    Qd = nc.dram_tensor("Q", [BPC, M, D], f32, kind="ExternalInput")
    w4Qd = nc.dram_tensor("w4Q", [D, 1], f32, kind="ExternalInput")
    wmlud = nc.dram_tensor("wmlu", [1, 1, D], f32, kind="ExternalInput")
    outd = nc.dram_tensor("out", [BPC, N, 4 * D], f32, kind="ExternalOutput")

    with tile.TileContext(nc) as tc:
        with (
            tc.tile_pool(name="consts", bufs=1) as consts,
            tc.tile_pool(name="cq", bufs=2) as cq,
            tc.tile_pool(name="work", bufs=1) as work,
            tc.tile_pool(name="stage", bufs=3) as stage,
            tc.tile_pool(name="ps_f", bufs=2, space="PSUM") as ps_f,
            tc.tile_pool(name="ps_acc", bufs=2, space="PSUM") as ps_acc,
            tc.tile_pool(name="ps_mid", bufs=2, space="PSUM") as ps_mid,
        ):
            ident = consts.tile([P, P], f32, tag="ident")
            make_identity(nc, ident)
            w4Qb = consts.tile([P, D], f32, tag="w4Qb")
            nc.gpsimd.dma_start(
                out=w4Qb, in_=bass.AP(tensor=w4Qd, offset=0, ap=[[0, P], [1, D]])
            )
            wmlu_pp = consts.tile([P, DC], f32, tag="wmlu")
            nc.gpsimd.dma_start(
                out=wmlu_pp, in_=bass.AP(tensor=wmlud, offset=0, ap=[[1, P], [P, DC]])
            )
            ones_f32 = consts.tile([P, 8], f32, tag="ones32")
            nc.vector.memset(ones_f32, 1.0)
            ones_col = consts.tile([P, 8], f32r, tag="ones")
            nc.vector.tensor_copy(out=ones_col, in_=ones_f32)

            for b in range(BPC):
                # ---- load ----
                C_nat = cq.tile([P, NC, D], f32, tag="c")
                nc.sync.dma_start(
                    out=C_nat, in_=Cd[b].rearrange("(c p) d -> p c d", p=P)
                )
                Q_nat = cq.tile([P, MC, D], f32, tag="q")
                nc.sync.dma_start(
                    out=Q_nat, in_=Qd[b].rearrange("(c p) d -> p c d", p=P)
                )

                if phase < 60:
                    for c in range(NC):
                        nc.sync.dma_start(
                            out=outd[b, c * P : (c + 1) * P, 0:512],
                            in_=C_nat[:, c, :],
                        )
                if phase < 20:
                    continue
                C_r = work.tile([P, NC, D], f32r, tag="cr")
                nc.vector.tensor_copy(out=C_r, in_=C_nat)
                Q_r = work.tile([P, MC, D], f32r, tag="qr")
                nc.vector.tensor_copy(out=Q_r, in_=Q_nat)
                CT = work.tile([P, DC, N], f32r, tag="ct")  # C^T: [d, n]
                QwT = work.tile([P, DC, M], f32r, tag="qwt")  # (Q*wmlu)^T: [d, m]
                ET = work.tile([P, MC, N], f32r, tag="et")  # E^T: [m, n]
                F = work.tile([P, NC, N], f32r, tag="f")  # diag(r)*F: [k, n]
                sub1 = work.tile([P, MC], f32, tag="sub1")
                rr = work.tile([P, NC], f32, tag="r")
                scr = work.tile([P, D], f32, tag="scr")

                # ---- transposes: CT[d, n], QwT[d, m] via PE ----
                for c in range(NC):
                    for e in range(DC):
                        tp = ps_mid.tile([P, P], f32, tag="mid")
                        nc.tensor.transpose(
                            tp, C_nat[:, c, e * P : (e + 1) * P], ident
                        )
                        nc.vector.tensor_copy(
                            out=CT[:, e, c * P : (c + 1) * P], in_=tp
                        )
                for mm in range(MC):
                    for e in range(DC):
                        tp = ps_mid.tile([P, P], f32, tag="mid")
                        nc.tensor.transpose(
                            tp, Q_nat[:, mm, e * P : (e + 1) * P], ident
                        )
                        # scale rows (=d) by w4mlu[d] while copying out of psum
                        nc.vector.tensor_scalar_mul(
                            out=QwT[:, e, mm * P : (mm + 1) * P],
                            in0=tp,
                            scalar1=wmlu_pp[:, e : e + 1],
                        )

                if phase < 30:
                    nc.sync.dma_start(
                        out=outd[b, 0:P, 512:1024],
                        in_=CT[:, 0, 0:512].bitcast(f32),
                    )
                    nc.sync.dma_start(
                        out=outd[b, 0:P, 1536:2048],
                        in_=QwT[:, 0, 0:512].bitcast(f32),
                    )
                    continue

                # ---- sub1[m] = sum_d Q[m,d] * w4Q[d]  (per-partition layout) ----
                for mm in range(MC):
                    nc.vector.tensor_mul(out=scr, in0=Q_nat[:, mm, :], in1=w4Qb)
                    nc.vector.reduce_sum(
                        out=sub1[:, mm : mm + 1],
                        in_=scr,
                        axis=mybir.AxisListType.X,
                    )

                if phase < 40:
                    nc.sync.dma_start(out=outd[b, 0:P, 516:520], in_=sub1)
                    continue

                # ---- (a) sim^T -> E^T = exp(sim^T + sub1) ----
                for mm in range(MC):
                    for nh in range(2):
                        sim_ps = ps_mid.tile([P, 512], f32, tag="mid")
                        for e in range(DC):
                            nc.tensor.matmul(
                                sim_ps,
                                lhsT=QwT[:, e, mm * P : (mm + 1) * P],
                                rhs=CT[:, e, nh * 512 : (nh + 1) * 512],
                                start=(e == 0),
                                stop=(e == DC - 1),
                            )
                        if phase < 50:
                            nc.vector.tensor_copy(
                                out=ET[:, mm, nh * 512 : (nh + 1) * 512],
                                in_=sim_ps,
                            )
                        else:
                            nc.scalar.activation(
                                out=ET[:, mm, nh * 512 : (nh + 1) * 512],
                                in_=sim_ps,
                                func=ACT.Exp,
                                bias=sub1[:, mm : mm + 1],
                                scale=1.0,
                            )

                if phase < 60:
                    nc.sync.dma_start(
                        out=outd[b, 0:P, 512:1024],
                        in_=ET[:, 0, 0:512].bitcast(f32),
                    )
                    continue

                # ---- fused loop over n-chunks: F row-block, A, rowsum ----
                for c in range(NC):
                    F_ps = ps_f.tile([P, N], f32, tag="f")
                    A_ps = ps_acc.tile([P, 512], f32, tag="acc")
                    rs_ps = ps_mid.tile([P, 8], f32, tag="mid")
                    for e in range(MC):
                        lhs = ET[:, e, c * P : (c + 1) * P]
                        st, sp = (e == 0), (e == MC - 1)
                        nc.tensor.matmul(
                            F_ps[:, 0:512],
                            lhsT=lhs,
                            rhs=ET[:, e, 0:512],
                            start=st,
                            stop=sp,
                        )
                        nc.tensor.matmul(
                            F_ps[:, 512:1024],
                            lhsT=lhs,
                            rhs=ET[:, e, 512:1024],
                            start=st,
                            stop=sp,
                        )
                        nc.tensor.matmul(
                            A_ps,
                            lhsT=lhs,
                            rhs=Q_r[:, e, :],
                            start=st,
                            stop=sp,
                        )
                        nc.tensor.matmul(
                            rs_ps,
                            lhsT=lhs,
                            rhs=ones_col,
                            start=st,
                            stop=sp,
                        )
                    # r[c] = 1 / rowsum
                    nc.vector.reciprocal(out=rr[:, c : c + 1], in_=rs_ps[:, 0:1])
                    # F'' = diag(r) F  (scalar engine, keeps DVE free)
                    nc.scalar.activation(
                        out=F[:, c, :],
                        in_=F_ps,
                        func=ACT.Copy,
                        scale=rr[:, c : c + 1],
                    )
                    # A = diag(r) (E Q)
                    A_s = stage.tile([P, 512], f32, tag="a")
                    nc.scalar.activation(
                        out=A_s, in_=A_ps, func=ACT.Copy, scale=rr[:, c : c + 1]
                    )
                    CA_s = stage.tile([P, 512], f32, tag="ca")
                    nc.vector.tensor_mul(out=CA_s, in0=C_nat[:, c, :], in1=A_s)
                    nc.sync.dma_start(
                        out=outd[b, c * P : (c + 1) * P, 0:512], in_=C_nat[:, c, :]
                    )
                    nc.sync.dma_start(
                        out=outd[b, c * P : (c + 1) * P, 512:1024], in_=A_s
                    )
                    nc.sync.dma_start(
                        out=outd[b, c * P : (c + 1) * P, 1024:1536], in_=CA_s
                    )

                if phase < 70:
                    nc.sync.dma_start(
                        out=outd[b, 0:P, 1536:2048],
                        in_=F[:, 0, 0:512].bitcast(f32),
                    )
                    continue

                # ---- (d) Bv = diag(r) sum_k F''[k, n] C[k, :] ----
                for c in range(NC):
                    Bv_ps = ps_acc.tile([P, 512], f32, tag="acc")
                    for i in range(NC):
                        nc.tensor.matmul(
                            Bv_ps,
                            lhsT=F[:, i, c * P : (c + 1) * P],
                            rhs=C_r[:, i, :],
                            start=(i == 0),
                            stop=(i == NC - 1),
                        )
                    Bv_s = stage.tile([P, 512], f32, tag="bv")
                    nc.scalar.activation(
                        out=Bv_s, in_=Bv_ps, func=ACT.Copy, scale=rr[:, c : c + 1]
                    )
                    CBv_s = stage.tile([P, 512], f32, tag="cbv")
                    nc.vector.tensor_mul(out=CBv_s, in0=C_nat[:, c, :], in1=Bv_s)
                    nc.sync.dma_start(
                        out=outd[b, c * P : (c + 1) * P, 1536:2048], in_=CBv_s
                    )

    nc.compile()
    return nc


def _reference_fallback(C, Q, Cmask, Qmask, w4C, w4Q, w4mlu, bias):
    """Numpy fallback for non-all-ones masks (not expected per spec)."""
    def softmax(x, axis):
        x = x - np.max(x, axis=axis, keepdims=True)
        e = np.exp(x)
        return e / np.sum(e, axis=axis, keepdims=True)

    sub0 = C @ w4C
    sub1 = np.swapaxes(Q @ w4Q, 1, 2)
    sub2 = np.einsum("bnd,bmd->bnm", C * w4mlu, Q)
    sim = sub0 + sub1 + sub2 + bias
    s1m = np.where(Qmask[:, None, :] == 0, -np.inf, sim)
    s2m = np.where(Cmask[:, :, None] == 0, -np.inf, sim)
    S1 = softmax(s1m, -1)
    S2 = softmax(s2m, -1)
    A = np.einsum("bnm,bmd->bnd", S1, Q)
    Bt = np.einsum("bnm,bkm->bnk", S1, S2)
    Bv = np.einsum("bnk,bkd->bnd", Bt, C)
    return np.concatenate([C, A, C * A, C * Bv], axis=2).astype(np.float32)


def kernel(C, Q, Cmask, Qmask, w4C, w4Q, w4mlu, bias):
    C = np.asarray(C, np.float32)
    Q = np.asarray(Q, np.float32)
    w4Q = np.asarray(w4Q, np.float32)
    w4mlu = np.asarray(w4mlu, np.float32)

    if not (np.all(np.asarray(Cmask) == 1) and np.all(np.asarray(Qmask) == 1)):
        return _reference_fallback(
            C, Q, np.asarray(Cmask), np.asarray(Qmask),
            np.asarray(w4C, np.float32), w4Q, w4mlu,
            np.asarray(bias, np.float32),
        )

    from concourse.bass_utils import run_bass_kernel_spmd
    import os

    if "nc" not in _cache:
        _cache["nc"] = _build()
    nc = _cache["nc"]

    in_maps = []
    for i in range(NCORES):
        in_maps.append(
            {
                "C": np.ascontiguousarray(C[i * BPC : (i + 1) * BPC]),
                "Q": np.ascontiguousarray(Q[i * BPC : (i + 1) * BPC]),
                "w4Q": np.ascontiguousarray(w4Q),
                "wmlu": np.ascontiguousarray(w4mlu),
            }
        )

    trace = bool(int(os.environ.get("BASS_KERNEL_TRACE", "0")))
    res = run_bass_kernel_spmd(
        nc, in_maps, core_ids=list(range(NCORES)), trace=trace
    )
    if trace:
        _cache["exec_time_ns"] = res.exec_time_ns
        _cache["trace"] = res.instructions_and_trace
    out = np.concatenate([r["out"] for r in res.results], axis=0)
    return out


# revision 12
# speedup vs baseline: 1.0596x; 1.0596x over previous
"""CQAttention Trainium2 kernel: out = concat([C, A, C*A, C*Bv], -1).

Math notes (exact, not approximations):
  - similarity = sub0 + sub1 + sub2 + bias is consumed only through
    softmax over the last axis (m). sub0 (per-row) and bias (scalar) are
    constant along m, so they cancel in both softmaxes -> dropped.
  - With all-ones masks S1 == S2, so Bt = S1 @ S1^T.
  - Softmax normalization is folded into per-partition scales:
      E^T[m,n] = exp(sub1[m] + sub2[n,m])   (no max-subtract; |scores| ~ 12)
      r[n] = 1 / sum_m E[n,m]
      A  = diag(r) (E Q)
      F  = E E^T (symmetric);  F'' = diag(r) F
      Bv = diag(r) (F''^T contracted with C) = diag(r) (sum_k F''[k,n] C[k,:])
  - All matmuls run as float32r (full-rate fp32 on trn2 PE).

Sharding: data-parallel over batch; core i handles batches [2i, 2i+1].
"""

import sys

if "/opt/trn_rl_repo" not in sys.path:
    sys.path.insert(0, "/opt/trn_rl_repo")

import numpy as np

B, N, M, D = 16, 1024, 512, 512
NCORES = 8
BPC = B // NCORES  # batches per core
P = 128
NC = N // P  # 8 n-chunks
MC = M // P  # 4 m-chunks
DC = D // P  # 4 d-chunks

_cache = {}


def _split_drain_waits(nc, max_waits=1):
    """This container's walrus rejects Drain instructions carrying more than
    ~1 sem wait. Hoist extra waits onto preceding single-wait Drains."""
    from concourse import mybir

    for f in nc.m.functions:
        for blk in f.blocks:
            out = []
            changed = False
            for inst in blk.instructions:
                si = inst.sync_info
                if (
                    getattr(inst, "opcode", None) == "Drain"
                    and si is not None
                    and si.on_wait is not None
                    and len(si.on_wait) > max_waits
                ):
                    waits = list(si.on_wait)
                    head, tail = waits[:-max_waits], waits[-max_waits:]
                    for j, w in enumerate(head):
                        out.append(
                            mybir.InstDrain(
                                name=f"{inst.name}-sw{j}",
                                engine=inst.engine,
                                ins=[],
                                outs=[],
                                sync_info=mybir.SyncInfo(on_wait=[w], on_update=[]),
                            )
                        )
                    inst.sync_info = mybir.SyncInfo(
                        on_wait=tail, on_update=list(si.on_update or [])
                    )
                    changed = True
                out.append(inst)
            if changed:
                blk.instructions = out


def _build(phase=70):
    import concourse.bass as bass
    import concourse.tile as tile
    from concourse import bacc, mybir
    from concourse.masks import make_identity

    f32 = mybir.dt.float32
    f32r = mybir.dt.float32r
    ACT = mybir.ActivationFunctionType

    nc = bacc.Bacc("TRN2")
    Cd = nc.dram_tensor("C", [BPC, N, D], f32, kind="ExternalInput")
    Qd = nc.dram_tensor("Q", [BPC, M, D], f32, kind="ExternalInput")
    w4Qd = nc.dram_tensor("w4Q", [D, 1], f32, kind="ExternalInput")
    wmlud = nc.dram_tensor("wmlu", [1, 1, D], f32, kind="ExternalInput")
    outd = nc.dram_tensor("out", [BPC, N, 4 * D], f32, kind="ExternalOutput")
    rs_dram = nc.dram_tensor("rs_scratch", [BPC, N], f32, kind="Internal")

    with tile.TileContext(nc) as tc:
        with (
            tc.tile_pool(name="consts", bufs=1) as consts,
            tc.tile_pool(name="cq", bufs=2) as cq,
            tc.tile_pool(name="work", bufs=1) as work,
            tc.tile_pool(name="stage", bufs=3) as stage,
            tc.tile_pool(name="ps_f", bufs=2, space="PSUM") as ps_f,
            tc.tile_pool(name="ps_acc", bufs=2, space="PSUM") as ps_acc,
            tc.tile_pool(name="ps_mid", bufs=2, space="PSUM") as ps_mid,
        ):
            ident = consts.tile([P, P], f32, tag="ident")
            make_identity(nc, ident)
            ident_r = consts.tile([P, P], f32r, tag="identr")
            nc.vector.tensor_copy(out=ident_r, in_=ident)
            w4Qb = consts.tile([P, D], f32, tag="w4Qb")
            nc.gpsimd.dma_start(
                out=w4Qb, in_=bass.AP(tensor=w4Qd, offset=0, ap=[[0, P], [1, D]])
            )
            wmlu_pp = consts.tile([P, DC], f32, tag="wmlu")
            nc.gpsimd.dma_start(
                out=wmlu_pp, in_=bass.AP(tensor=wmlud, offset=0, ap=[[1, P], [P, DC]])
            )
            ones_f32 = consts.tile([P, 8], f32, tag="ones32")
            nc.vector.memset(ones_f32, 1.0)
            ones_col = consts.tile([P, 8], f32r, tag="ones")
            nc.vector.tensor_copy(out=ones_col, in_=ones_f32)

            for b in range(BPC):
                # ---- loads: f32r copies (DMA-cast) first so transposes start
                # early; exact-f32 copies trail (needed later) ----
                C_r = work.tile([P, NC, D], f32r, tag="cr")
                Q_r = work.tile([P, MC, D], f32r, tag="qr")
                for c in range(NC):
                    nc.gpsimd.dma_start(
                        out=C_r[:, c, :], in_=Cd[b, c * P : (c + 1) * P, :]
                    )
                for mm in range(MC):
                    nc.gpsimd.dma_start(
                        out=Q_r[:, mm, :], in_=Qd[b, mm * P : (mm + 1) * P, :]
                    )
                C_nat = cq.tile([P, NC, D], f32, tag="c")
                Q_nat = cq.tile([P, MC, D], f32, tag="q")
                for c in range(NC):
                    nc.sync.dma_start(
                        out=C_nat[:, c, :], in_=Cd[b, c * P : (c + 1) * P, :]
                    )
                for mm in range(MC):
                    nc.sync.dma_start(
                        out=Q_nat[:, mm, :], in_=Qd[b, mm * P : (mm + 1) * P, :]
                    )
                for c in range(NC):
                    nc.sync.dma_start(
                        out=outd[b, c * P : (c + 1) * P, 0:512],
                        in_=C_nat[:, c, :],
                    )
                if phase < 20:
                    continue
                CT = work.tile([P, DC, N], f32r, tag="ct")  # C^T: [d, n]
                QwT = work.tile([P, DC, M], f32r, tag="qwt")  # (Q*wmlu)^T: [d, m]
                ET = work.tile([P, MC, N], f32r, tag="et")  # E^T: [m, n]
                F = work.tile([P, NC, N], f32r, tag="f")  # diag(r)*F: [k, n]
                sub1 = work.tile([P, MC], f32, tag="sub1")
                rr = work.tile([P, NC], f32, tag="r")
                scr = work.tile([P, D], f32, tag="scr")

                # ---- transposes: CT[d, n], QwT[d, m] via PE (f32r, 1.5cyc/row) ----
                for c in range(NC):
                    for e in range(DC):
                        pool = ps_mid if (c * DC + e) % 2 == 0 else ps_acc
                        tag = "mid" if (c * DC + e) % 2 == 0 else "acc"
                        tp = pool.tile([P, P], f32r, tag=tag)
                        nc.tensor.transpose(
                            tp, C_r[:, c, e * P : (e + 1) * P], ident_r
                        )
                        nc.vector.tensor_copy(
                            out=CT[:, e, c * P : (c + 1) * P], in_=tp
                        )
                for mm in range(MC):
                    for e in range(DC):
                        pool = ps_mid if (mm * DC + e) % 2 == 0 else ps_acc
                        tag = "mid" if (mm * DC + e) % 2 == 0 else "acc"
                        tp = pool.tile([P, P], f32r, tag=tag)
                        nc.tensor.transpose(
                            tp, Q_r[:, mm, e * P : (e + 1) * P], ident_r
                        )
                        # scale rows (=d) by w4mlu[d] while copying out of psum
                        nc.scalar.activation(
                            out=QwT[:, e, mm * P : (mm + 1) * P],
                            in_=tp,
                            func=ACT.Copy,
                            scale=wmlu_pp[:, e : e + 1],
                        )

                if phase < 30:
                    nc.sync.dma_start(
                        out=outd[b, 0:P, 512:1024],
                        in_=CT[:, 0, 0:512].bitcast(f32),
                    )
                    nc.sync.dma_start(
                        out=outd[b, 0:P, 1536:2048],
                        in_=QwT[:, 0, 0:512].bitcast(f32),
                    )
                    continue

                # ---- sub1[m] = sum_d Q[m,d] * w4Q[d]  (per-partition layout) ----
                for mm in range(MC):
                    nc.vector.tensor_mul(out=scr, in0=Q_nat[:, mm, :], in1=w4Qb)
                    nc.vector.reduce_sum(
                        out=sub1[:, mm : mm + 1],
                        in_=scr,
                        axis=mybir.AxisListType.X,
                    )

                if phase < 40:
                    nc.sync.dma_start(out=outd[b, 0:P, 516:520], in_=sub1)
                    continue

                # ---- (a) sim^T -> E^T = exp(sim^T + sub1) ----
                for mm in range(MC):
                    for nh in range(2):
                        sim_ps = ps_mid.tile([P, 512], f32, tag="mid")
                        for e in range(DC):
                            nc.tensor.matmul(
                                sim_ps,
                                lhsT=QwT[:, e, mm * P : (mm + 1) * P],
                                rhs=CT[:, e, nh * 512 : (nh + 1) * 512],
                                start=(e == 0),
                                stop=(e == DC - 1),
                            )
                        if phase < 50:
                            nc.vector.tensor_copy(
                                out=ET[:, mm, nh * 512 : (nh + 1) * 512],
                                in_=sim_ps,
                            )
                        else:
                            nc.scalar.activation(
                                out=ET[:, mm, nh * 512 : (nh + 1) * 512],
                                in_=sim_ps,
                                func=ACT.Exp,
                                bias=sub1[:, mm : mm + 1],
                                scale=1.0,
                            )

                # ---- rowsum: rsT[j, n] = sum_m E^T[m, n] via ones weights ----
                rs_row = work.tile([1, N], f32, tag="rsrow")
                for nh in range(2):
                    rsT_ps = ps_mid.tile([8, 512], f32, tag="mid")
                    for e in range(MC):
                        nc.tensor.matmul(
                            rsT_ps,
                            lhsT=ones_col,
                            rhs=ET[:, e, nh * 512 : (nh + 1) * 512],
                            start=(e == 0),
                            stop=(e == MC - 1),
                        )
                    nc.vector.tensor_copy(
                        out=rs_row[:, nh * 512 : (nh + 1) * 512], in_=rsT_ps[0:1, :]
                    )
                # re-layout [1, N] -> per-partition [P, NC] via DRAM bounce
                nc.sync.dma_start(out=rs_dram[b], in_=rs_row[0:1, :])
                rs_pp = work.tile([P, NC], f32, tag="rspp")
                nc.sync.dma_start(
                    out=rs_pp, in_=rs_dram[b].rearrange("(c p) -> p c", p=P)
                )
                nc.vector.reciprocal(out=rr, in_=rs_pp)

                if phase < 60:
                    nc.sync.dma_start(
                        out=outd[b, 0:P, 512:1024],
                        in_=ET[:, 0, 0:512].bitcast(f32),
                    )
                    nc.sync.dma_start(out=outd[b, 0:P, 1032:1040], in_=rr)
                    continue

                # ---- fused loop over n-chunks: F row-block, A ----
                for c in range(NC):
                    F_ps = ps_f.tile([P, N], f32, tag="f")
                    A_ps = ps_acc.tile([P, 512], f32, tag="acc")
                    for e in range(MC):
                        lhs = ET[:, e, c * P : (c + 1) * P]
                        st, sp = (e == 0), (e == MC - 1)
                        nc.tensor.matmul(
                            F_ps[:, 0:512],
                            lhsT=lhs,
                            rhs=ET[:, e, 0:512],
                            start=st,
                            stop=sp,
                        )
                        nc.tensor.matmul(
                            F_ps[:, 512:1024],
                            lhsT=lhs,
                            rhs=ET[:, e, 512:1024],
                            start=st,
                            stop=sp,
                        )
                        nc.tensor.matmul(
                            A_ps,
                            lhsT=lhs,
                            rhs=Q_r[:, e, :],
                            start=st,
                            stop=sp,
                        )
                    # F'' = diag(r) F  (scalar engine, keeps DVE free)
                    nc.scalar.activation(
                        out=F[:, c, :],
                        in_=F_ps,
                        func=ACT.Copy,
                        scale=rr[:, c : c + 1],
                    )
                    # A = diag(r) (E Q)
                    A_s = stage.tile([P, 512], f32, tag="a")
                    nc.scalar.activation(
                        out=A_s, in_=A_ps, func=ACT.Copy, scale=rr[:, c : c + 1]
                    )
                    CA_s = stage.tile([P, 512], f32, tag="ca")
                    nc.vector.tensor_mul(out=CA_s, in0=C_nat[:, c, :], in1=A_s)
                    nc.sync.dma_start(
                        out=outd[b, c * P : (c + 1) * P, 0:512], in_=C_nat[:, c, :]
                    )
                    nc.sync.dma_start(
                        out=outd[b, c * P : (c + 1) * P, 512:1024], in_=A_s
                    )
                    nc.sync.dma_start(
                        out=outd[b, c * P : (c + 1) * P, 1024:1536], in_=CA_s
                    )

                if phase < 70:
                    nc.sync.dma_start(
                        out=outd[b, 0:P, 1536:2048],
                        in_=F[:, 0, 0:512].bitcast(f32),
                    )
                    continue

                # ---- (d) Bv = diag(r) sum_k F''[k, n] C[k, :] ----
                for c in range(NC):
                    Bv_ps = ps_acc.tile([P, 512], f32, tag="acc")
                    for i in range(NC):
                        nc.tensor.matmul(
                            Bv_ps,
                            lhsT=F[:, i, c * P : (c + 1) * P],
                            rhs=C_r[:, i, :],
                            start=(i == 0),
                            stop=(i == NC - 1),
                        )
                    Bv_s = stage.tile([P, 512], f32, tag="bv")
                    nc.scalar.activation(
                        out=Bv_s, in_=Bv_ps, func=ACT.Copy, scale=rr[:, c : c + 1]
                    )
                    CBv_s = stage.tile([P, 512], f32, tag="cbv")
                    nc.vector.tensor_mul(out=CBv_s, in0=C_nat[:, c, :], in1=Bv_s)
                    nc.sync.dma_start(
                        out=outd[b, c * P : (c + 1) * P, 1536:2048], in_=CBv_s
                    )

    nc.compile()
    return nc


def _reference_fallback(C, Q, Cmask, Qmask, w4C, w4Q, w4mlu, bias):
    """Numpy fallback for non-all-ones masks (not expected per spec)."""
    def softmax(x, axis):
        x = x - np.max(x, axis=axis, keepdims=True)
        e = np.exp(x)
        return e / np.sum(e, axis=axis, keepdims=True)

    sub0 = C @ w4C
    sub1 = np.swapaxes(Q @ w4Q, 1, 2)
    sub2 = np.einsum("bnd,bmd->bnm", C * w4mlu, Q)
    sim = sub0 + sub1 + sub2 + bias
    s1m = np.where(Qmask[:, None, :] == 0, -np.inf, sim)
    s2m = np.where(Cmask[:, :, None] == 0, -np.inf, sim)
    S1 = softmax(s1m, -1)
    S2 = softmax(s2m, -1)
    A = np.einsum("bnm,bmd->bnd", S1, Q)
    Bt = np.einsum("bnm,bkm->bnk", S1, S2)
    Bv = np.einsum("bnk,bkd->bnd", Bt, C)
    return np.concatenate([C, A, C * A, C * Bv], axis=2).astype(np.float32)


def kernel(C, Q, Cmask, Qmask, w4C, w4Q, w4mlu, bias):
    C = np.asarray(C, np.float32)
    Q = np.asarray(Q, np.float32)
    w4Q = np.asarray(w4Q, np.float32)
    w4mlu = np.asarray(w4mlu, np.float32)

    if not (np.all(np.asarray(Cmask) == 1) and np.all(np.asarray(Qmask) == 1)):
        return _reference_fallback(
            C, Q, np.asarray(Cmask), np.asarray(Qmask),
            np.asarray(w4C, np.float32), w4Q, w4mlu,
            np.asarray(bias, np.float32),
        )

    from concourse.bass_utils import run_bass_kernel_spmd
    import os

    if "nc" not in _cache:
        _cache["nc"] = _build()
    nc = _cache["nc"]

    in_maps = []
    for i in range(NCORES):
        in_maps.append(
            {
                "C": np.ascontiguousarray(C[i * BPC : (i + 1) * BPC]),
                "Q": np.ascontiguousarray(Q[i * BPC : (i + 1) * BPC]),
                "w4Q": np.ascontiguousarray(w4Q),
                "wmlu": np.ascontiguousarray(w4mlu),
            }
        )

    trace = bool(int(os.environ.get("BASS_KERNEL_TRACE", "0")))
    res = run_bass_kernel_spmd(
        nc, in_maps, core_ids=list(range(NCORES)), trace=trace
    )
    if trace:
        _cache["exec_time_ns"] = res.exec_time_ns
        _cache["trace"] = res.instructions_and_trace
    out = np.concatenate([r["out"] for r in res.results], axis=0)
    return out


# revision 14
# speedup vs baseline: 1.1737x; 1.1076x over previous
"""CQAttention Trainium2 kernel: out = concat([C, A, C*A, C*Bv], -1).

Math notes (exact, not approximations):
  - similarity = sub0 + sub1 + sub2 + bias is consumed only through
    softmax over the last axis (m). sub0 (per-row) and bias (scalar) are
    constant along m, so they cancel in both softmaxes -> dropped.
  - With all-ones masks S1 == S2, so Bt = S1 @ S1^T.
  - Softmax normalization is folded into per-partition scales:
      E^T[m,n] = exp(sub1[m] + sub2[n,m])   (no max-subtract; |scores| ~ 12)
      r[n] = 1 / sum_m E[n,m]
      A  = diag(r) (E Q)
      F  = E E^T (symmetric);  F'' = diag(r) F
      Bv = diag(r) (F''^T contracted with C) = diag(r) (sum_k F''[k,n] C[k,:])
  - All matmuls run as float32r (full-rate fp32 on trn2 PE).

Sharding: data-parallel over batch; core i handles batches [2i, 2i+1].
"""

import sys

if "/opt/trn_rl_repo" not in sys.path:
    sys.path.insert(0, "/opt/trn_rl_repo")

import numpy as np

B, N, M, D = 16, 1024, 512, 512
NCORES = 8
BPC = B // NCORES  # batches per core
P = 128
NC = N // P  # 8 n-chunks
MC = M // P  # 4 m-chunks
DC = D // P  # 4 d-chunks

_cache = {}


def _split_drain_waits(nc, max_waits=1):
    """This container's walrus rejects Drain instructions carrying more than
    ~1 sem wait. Hoist extra waits onto preceding single-wait Drains."""
    from concourse import mybir

    for f in nc.m.functions:
        for blk in f.blocks:
            out = []
            changed = False
            for inst in blk.instructions:
                si = inst.sync_info
                if (
                    getattr(inst, "opcode", None) == "Drain"
                    and si is not None
                    and si.on_wait is not None
                    and len(si.on_wait) > max_waits
                ):
                    waits = list(si.on_wait)
                    head, tail = waits[:-max_waits], waits[-max_waits:]
                    for j, w in enumerate(head):
                        out.append(
                            mybir.InstDrain(
                                name=f"{inst.name}-sw{j}",
                                engine=inst.engine,
                                ins=[],
                                outs=[],
                                sync_info=mybir.SyncInfo(on_wait=[w], on_update=[]),
                            )
                        )
                    inst.sync_info = mybir.SyncInfo(
                        on_wait=tail, on_update=list(si.on_update or [])
                    )
                    changed = True
                out.append(inst)
            if changed:
                blk.instructions = out


def _build(phase=70):
    import concourse.bass as bass
    import concourse.tile as tile
    from concourse import bacc, mybir
    from concourse.masks import make_identity

    f32 = mybir.dt.float32
    f32r = mybir.dt.float32r
    ACT = mybir.ActivationFunctionType

    nc = bacc.Bacc("TRN2")
    Cd = nc.dram_tensor("C", [BPC, N, D], f32, kind="ExternalInput")
    Qd = nc.dram_tensor("Q", [BPC, M, D], f32, kind="ExternalInput")
    w4Qd = nc.dram_tensor("w4Q", [D, 1], f32, kind="ExternalInput")
    wmlud = nc.dram_tensor("wmlu", [1, 1, D], f32, kind="ExternalInput")
    outd = nc.dram_tensor("out", [BPC, N, 4 * D], f32, kind="ExternalOutput")
    rs_dram = nc.dram_tensor("rs_scratch", [BPC, N], f32, kind="Internal")

    with tile.TileContext(nc) as tc:
        with (
            tc.tile_pool(name="consts", bufs=1) as consts,
            tc.tile_pool(name="cq", bufs=2) as cq,
            tc.tile_pool(name="work", bufs=1) as work,
            tc.tile_pool(name="stage", bufs=3) as stage,
            tc.tile_pool(name="ps_f", bufs=2, space="PSUM") as ps_f,
            tc.tile_pool(name="ps_acc", bufs=2, space="PSUM") as ps_acc,
            tc.tile_pool(name="ps_mid", bufs=2, space="PSUM") as ps_mid,
        ):
            ident = consts.tile([P, P], f32, tag="ident")
            make_identity(nc, ident)
            ident_r = consts.tile([P, P], f32r, tag="identr")
            nc.vector.tensor_copy(out=ident_r, in_=ident)
            w4Qb = consts.tile([P, D], f32, tag="w4Qb")
            nc.gpsimd.dma_start(
                out=w4Qb, in_=bass.AP(tensor=w4Qd, offset=0, ap=[[0, P], [1, D]])
            )
            wmlu_pp = consts.tile([P, DC], f32, tag="wmlu")
            nc.gpsimd.dma_start(
                out=wmlu_pp, in_=bass.AP(tensor=wmlud, offset=0, ap=[[1, P], [P, DC]])
            )
            ones_f32 = consts.tile([P, 8], f32, tag="ones32")
            nc.vector.memset(ones_f32, 1.0)
            ones_col = consts.tile([P, 8], f32r, tag="ones")
            nc.vector.tensor_copy(out=ones_col, in_=ones_f32)

            for b in range(BPC):
                # ---- loads: C/Q as f32r-tagged byte copies (HWDGE, no cast;
                # f32r SBUF bits are the exact f32 bits) ----
                C_r = cq.tile([P, NC, D], f32r, tag="cr")
                Q_r = cq.tile([P, MC, D], f32r, tag="qr")
                for c in range(NC):
                    nc.sync.dma_start(
                        out=C_r[:, c, :],
                        in_=Cd[b, c * P : (c + 1) * P, :].bitcast(f32r),
                    )
                for mm in range(MC):
                    nc.sync.dma_start(
                        out=Q_r[:, mm, :],
                        in_=Qd[b, mm * P : (mm + 1) * P, :].bitcast(f32r),
                    )
                # C passthrough (exact bytes) straight from C_r
                nc.sync.dma_start(
                    out=outd[b, :, 0:512].rearrange("(c p) d -> p c d", p=P),
                    in_=C_r.bitcast(f32),
                )
                if phase < 20:
                    continue
                CT = work.tile([P, DC, N], f32r, tag="ct")  # C^T: [d, n]
                QwT = work.tile([P, DC, M], f32r, tag="qwt")  # (Q*wmlu)^T: [d, m]
                ET = work.tile([P, MC, N], f32r, tag="et")  # E^T: [m, n]
                F = work.tile([P, NC, N], f32r, tag="f")  # diag(r)*F: [k, n]
                sub1 = work.tile([P, MC], f32, tag="sub1")
                rr = work.tile([P, NC], f32, tag="r")
                scr = work.tile([P, D], f32, tag="scr")

                # ---- transposes: CT[d, n], QwT[d, m] via PE (f32r, 1.5cyc/row) ----
                for c in range(NC):
                    for e in range(DC):
                        pool = ps_mid if (c * DC + e) % 2 == 0 else ps_acc
                        tag = "mid" if (c * DC + e) % 2 == 0 else "acc"
                        tp = pool.tile([P, P], f32r, tag=tag)
                        nc.tensor.transpose(
                            tp, C_r[:, c, e * P : (e + 1) * P], ident_r
                        )
                        nc.vector.tensor_copy(
                            out=CT[:, e, c * P : (c + 1) * P], in_=tp
                        )
                for mm in range(MC):
                    for e in range(DC):
                        pool = ps_mid if (mm * DC + e) % 2 == 0 else ps_acc
                        tag = "mid" if (mm * DC + e) % 2 == 0 else "acc"
                        tp = pool.tile([P, P], f32r, tag=tag)
                        nc.tensor.transpose(
                            tp, Q_r[:, mm, e * P : (e + 1) * P], ident_r
                        )
                        # scale rows (=d) by w4mlu[d] while copying out of psum
                        nc.vector.tensor_scalar_mul(
                            out=QwT[:, e, mm * P : (mm + 1) * P],
                            in0=tp,
                            scalar1=wmlu_pp[:, e : e + 1],
                        )

                if phase < 30:
                    nc.sync.dma_start(
                        out=outd[b, 0:P, 512:1024],
                        in_=CT[:, 0, 0:512].bitcast(f32),
                    )
                    nc.sync.dma_start(
                        out=outd[b, 0:P, 1536:2048],
                        in_=QwT[:, 0, 0:512].bitcast(f32),
                    )
                    continue

                # ---- sub1[m] = sum_d Q[m,d] * w4Q[d]  (per-partition layout) ----
                for mm in range(MC):
                    nc.vector.tensor_mul(
                        out=scr, in0=Q_r[:, mm, :].bitcast(f32), in1=w4Qb
                    )
                    nc.vector.reduce_sum(
                        out=sub1[:, mm : mm + 1],
                        in_=scr,
                        axis=mybir.AxisListType.X,
                    )

                if phase < 40:
                    nc.sync.dma_start(out=outd[b, 0:P, 516:520], in_=sub1)
                    continue

                # ---- (a) sim^T -> E^T = exp(sim^T + sub1); rowsum per half ----
                rs_row = work.tile([1, N], f32, tag="rsrow")
                for nh in range(2):
                    for mm in range(MC):
                        sim_ps = ps_mid.tile([P, 512], f32, tag="mid")
                        for e in range(DC):
                            nc.tensor.matmul(
                                sim_ps,
                                lhsT=QwT[:, e, mm * P : (mm + 1) * P],
                                rhs=CT[:, e, nh * 512 : (nh + 1) * 512],
                                start=(e == 0),
                                stop=(e == DC - 1),
                            )
                        if phase < 50:
                            nc.vector.tensor_copy(
                                out=ET[:, mm, nh * 512 : (nh + 1) * 512],
                                in_=sim_ps,
                            )
                        else:
                            nc.scalar.activation(
                                out=ET[:, mm, nh * 512 : (nh + 1) * 512],
                                in_=sim_ps,
                                func=ACT.Exp,
                                bias=sub1[:, mm : mm + 1],
                                scale=1.0,
                            )
                    if phase >= 50:
                        rsT_ps = ps_acc.tile([8, 512], f32, tag="acc")
                        for e in range(MC):
                            nc.tensor.matmul(
                                rsT_ps,
                                lhsT=ones_col,
                                rhs=ET[:, e, nh * 512 : (nh + 1) * 512],
                                start=(e == 0),
                                stop=(e == MC - 1),
                            )
                        nc.vector.tensor_copy(
                            out=rs_row[:, nh * 512 : (nh + 1) * 512],
                            in_=rsT_ps[0:1, :],
                        )

                # re-layout [1, N] -> per-partition [P, NC] via DRAM bounce
                nc.sync.dma_start(out=rs_dram[b], in_=rs_row[0:1, :])
                rs_pp = work.tile([P, NC], f32, tag="rspp")
                nc.sync.dma_start(
                    out=rs_pp, in_=rs_dram[b].rearrange("(c p) -> p c", p=P)
                )
                nc.vector.reciprocal(out=rr, in_=rs_pp)

                if phase < 60:
                    nc.sync.dma_start(
                        out=outd[b, 0:P, 512:1024],
                        in_=ET[:, 0, 0:512].bitcast(f32),
                    )
                    nc.sync.dma_start(out=outd[b, 0:P, 1032:1040], in_=rr)
                    continue

                # ---- fused loop over n-chunks: F row-block, A ----
                for c in range(NC):
                    F_ps = ps_f.tile([P, N], f32, tag="f")
                    A_ps = ps_acc.tile([P, 512], f32, tag="acc")
                    for e in range(MC):
                        lhs = ET[:, e, c * P : (c + 1) * P]
                        st, sp = (e == 0), (e == MC - 1)
                        nc.tensor.matmul(
                            F_ps[:, 0:512],
                            lhsT=lhs,
                            rhs=ET[:, e, 0:512],
                            start=st,
                            stop=sp,
                        )
                        nc.tensor.matmul(
                            F_ps[:, 512:1024],
                            lhsT=lhs,
                            rhs=ET[:, e, 512:1024],
                            start=st,
                            stop=sp,
                        )
                        nc.tensor.matmul(
                            A_ps,
                            lhsT=lhs,
                            rhs=Q_r[:, e, :],
                            start=st,
                            stop=sp,
                        )
                    # F'' = diag(r) F  (scalar engine, keeps DVE free)
                    nc.scalar.activation(
                        out=F[:, c, :],
                        in_=F_ps,
                        func=ACT.Copy,
                        scale=rr[:, c : c + 1],
                    )
                    # A = diag(r) (E Q)
                    A_s = stage.tile([P, 512], f32, tag="a")
                    nc.scalar.activation(
                        out=A_s, in_=A_ps, func=ACT.Copy, scale=rr[:, c : c + 1]
                    )
                    CA_s = stage.tile([P, 512], f32, tag="ca")
                    nc.vector.tensor_mul(
                        out=CA_s, in0=C_r[:, c, :].bitcast(f32), in1=A_s
                    )
                    nc.sync.dma_start(
                        out=outd[b, c * P : (c + 1) * P, 512:1024], in_=A_s
                    )
                    nc.sync.dma_start(
                        out=outd[b, c * P : (c + 1) * P, 1024:1536], in_=CA_s
                    )

                if phase < 70:
                    nc.sync.dma_start(
                        out=outd[b, 0:P, 1536:2048],
                        in_=F[:, 0, 0:512].bitcast(f32),
                    )
                    continue

                # ---- (d) Bv = diag(r) sum_k F''[k, n] C[k, :] ----
                for c in range(NC):
                    Bv_ps = ps_acc.tile([P, 512], f32, tag="acc")
                    for i in range(NC):
                        nc.tensor.matmul(
                            Bv_ps,
                            lhsT=F[:, i, c * P : (c + 1) * P],
                            rhs=C_r[:, i, :],
                            start=(i == 0),
                            stop=(i == NC - 1),
                        )
                    Bv_s = stage.tile([P, 512], f32, tag="bv")
                    nc.scalar.activation(
                        out=Bv_s, in_=Bv_ps, func=ACT.Copy, scale=rr[:, c : c + 1]
                    )
                    CBv_s = stage.tile([P, 512], f32, tag="cbv")
                    nc.vector.tensor_mul(
                        out=CBv_s, in0=C_r[:, c, :].bitcast(f32), in1=Bv_s
                    )
                    nc.sync.dma_start(
                        out=outd[b, c * P : (c + 1) * P, 1536:2048], in_=CBv_s
                    )

    nc.compile()
    return nc


def _reference_fallback(C, Q, Cmask, Qmask, w4C, w4Q, w4mlu, bias):
    """Numpy fallback for non-all-ones masks (not expected per spec)."""
    def softmax(x, axis):
        x = x - np.max(x, axis=axis, keepdims=True)
        e = np.exp(x)
        return e / np.sum(e, axis=axis, keepdims=True)

    sub0 = C @ w4C
    sub1 = np.swapaxes(Q @ w4Q, 1, 2)
    sub2 = np.einsum("bnd,bmd->bnm", C * w4mlu, Q)
    sim = sub0 + sub1 + sub2 + bias
    s1m = np.where(Qmask[:, None, :] == 0, -np.inf, sim)
    s2m = np.where(Cmask[:, :, None] == 0, -np.inf, sim)
    S1 = softmax(s1m, -1)
    S2 = softmax(s2m, -1)
    A = np.einsum("bnm,bmd->bnd", S1, Q)
    Bt = np.einsum("bnm,bkm->bnk", S1, S2)
    Bv = np.einsum("bnk,bkd->bnd", Bt, C)
    return np.concatenate([C, A, C * A, C * Bv], axis=2).astype(np.float32)


def kernel(C, Q, Cmask, Qmask, w4C, w4Q, w4mlu, bias):
    C = np.asarray(C, np.float32)
    Q = np.asarray(Q, np.float32)
    w4Q = np.asarray(w4Q, np.float32)
    w4mlu = np.asarray(w4mlu, np.float32)

    if not (np.all(np.asarray(Cmask) == 1) and np.all(np.asarray(Qmask) == 1)):
        return _reference_fallback(
            C, Q, np.asarray(Cmask), np.asarray(Qmask),
            np.asarray(w4C, np.float32), w4Q, w4mlu,
            np.asarray(bias, np.float32),
        )

    from concourse.bass_utils import run_bass_kernel_spmd
    import os

    if "nc" not in _cache:
        _cache["nc"] = _build()
    nc = _cache["nc"]

    in_maps = []
    for i in range(NCORES):
        in_maps.append(
            {
                "C": np.ascontiguousarray(C[i * BPC : (i + 1) * BPC]),
                "Q": np.ascontiguousarray(Q[i * BPC : (i + 1) * BPC]),
                "w4Q": np.ascontiguousarray(w4Q),
                "wmlu": np.ascontiguousarray(w4mlu),
            }
        )

    trace = bool(int(os.environ.get("BASS_KERNEL_TRACE", "0")))
    res = run_bass_kernel_spmd(
        nc, in_maps, core_ids=list(range(NCORES)), trace=trace
    )
    if trace:
        _cache["exec_time_ns"] = res.exec_time_ns
        _cache["trace"] = res.instructions_and_trace
    out = np.concatenate([r["out"] for r in res.results], axis=0)
    return out
